# revision 18
# baseline (speedup 1.0000x reference)
"""DLSMN scatter-memory + cache self-attention kernel for Trainium2.

Data-parallel over batch: batch b runs on NeuronCore b (8 cores), no
collectives.  Per-instruction overhead dominates on this part, so the
design minimizes instruction counts: DMA-crossbar transposes instead of
PE transposes, merged weight blocks, fp8 DoubleRow matmuls, CL=512
attention passes over head pairs, and the softmax exp stream split
across ScalarE / VectorE / Pool (the latter two via a Schraudolph-style
int8-bitcast exp that lands directly in fp8e4m3).

  phase A: per 128-token y tile: y -> bf16 (Pool), DMA-crossbar
           transpose -> ACT cast to fp8, 4 fp8 DoubleRow matmuls
           against the merged [W_write | W_slot | W_gate x2] block +
           one bias-row matmul, gumbel-softmax routing, bf16 scatter
           matmuls with the mass column folded into the update rhs.
           Cache tiles stream in f32, cast to bf16 on DVE, and are
           DMA-transposed into cache2T during the loop.
  phase B: slot update  upd = (1-g)*DECAY*old + g*updates/(mass+eps).
  phase C: only the 2 rewritten tiles' transposes + bf16->fp8 casts of
           cache2T (everything else already done during phase A).
  phase D: q/k/v projections with fp8 DoubleRow; q/k evacuate to an
           fp8 staging buffer, then SBUF->SBUF DMAs fold [128,h,n] ->
           [64,2,h,n] so QK^T runs as fp8 DoubleRow (hd split across
           partition halves).
  phase E: 4 chunks of 512 queries x 2 head-pair passes.  Per pass:
           16 QK DoubleRow matmuls -> exp into fp8 pT (10 on ScalarE,
           3 on DVE + 3 on Pool via int8 Schraudolph), PV DoubleRow
           pairs + one h-paired DoubleRow denominator matmul per pair,
           previous pass's tail + normalize overlapped.
  phase F: pipelined one chunk behind: o-projection fp8 DoubleRow,
           residual + layernorm with a DVE-only Quake rsqrt, output
           DMA per n-tile.
"""

import numpy as np

import concourse.bacc as bacc
import concourse.mybir as mybir
import concourse.tile as tile
from concourse.bass_utils import run_bass_kernel_spmd

F32 = mybir.dt.float32
F16 = mybir.dt.float16
BF16 = mybir.dt.bfloat16
FP8 = mybir.dt.float8e4
I32 = mybir.dt.int32
I8 = mybir.dt.int8
AF = mybir.ActivationFunctionType
ALU = mybir.AluOpType
DR = mybir.MatmulPerfMode.DoubleRow

B = 8
S = 2048
D = 1024
DC = 512
K = 256
L = 8
H = 4
HD = 128
N = L * K
LAYER_IDX = 3
DECAY = 0.9
EPS = 1e-6
ST = S // 128   # 16 token tiles
NT = N // 128   # 16 slot tiles
DCH = D // 128  # 8 d_model chunks
CL = 512        # attention n-chunk length (per pass: 2 heads)
NCH = N // CL   # 4 attention chunks
WA = DC + K + 2  # merged A-phase weight width: write | slot | gate gate
ATT_SCALE = float(1.0 / np.sqrt(np.float32(HD)))
QMAGIC = 0x5F3759DF + 1  # quake rsqrt magic (+1 for the xor-negate trick)
# Schraudolph int8 exp: i8 = x*ATT_SCALE*8/ln2 + B8, bitcast fp8e4m3
SCH_A = float(ATT_SCALE * 8.0 / np.log(2.0))
SCH_B = 55.75
# exp engine assignment per m-tile: 10 ScalarE, 6 DVE (Pool cannot
# read PSUM on TRN2)
EXP_ENG = ['a', 'a', 'a', 'v', 'a', 'a', 'v', 'a', 'a', 'v', 'a', 'v',
           'a', 'v', 'a', 'v']

_INPUT_SPECS = {
    "y": (S, D), "cache": (N, DC), "gumbel_u": (S, K),
    "W_gate": (D, 1), "b_gate": (1,), "W_slot": (D, K), "b_slot": (K,),
    "gamma": (1,), "W_write": (D, DC), "b_write": (DC,),
    "Wq": (DC, DC), "bq": (DC,), "Wk": (DC, DC), "bk": (DC,),
    "Wv": (DC, DC), "bv": (DC,), "Wo": (DC, DC), "bo": (DC,),
    "ln_g": (DC,), "ln_b": (DC,),
}


def _build():
    nc = bacc.Bacc("TRN2", target_bir_lowering=False, debug=False, num_devices=B)

    a = {
        name: nc.dram_tensor(name, list(shape), F32, kind="ExternalInput").ap()
        for name, shape in _INPUT_SPECS.items()
    }
    out_dram = nc.dram_tensor("out", [N, DC], F32, kind="ExternalOutput").ap()

    y3 = a["y"].rearrange("(t p) d -> p t d", p=128)
    gum3 = a["gumbel_u"].rearrange("(t p) k -> p t k", p=128)
    cache3 = a["cache"].rearrange("(t p) d -> p t d", p=128)
    out3 = out_dram.rearrange("(t p) d -> p t d", p=128)

    with tile.TileContext(nc) as tc:
        with (
            tc.tile_pool(name="const", bufs=1) as const,
            tc.tile_pool(name="cachep", bufs=1) as cachep,
            tc.tile_pool(name="attn", bufs=1) as attn,
        ):
            # ---------------- constants ------------------------------------
            ones_row_bf = const.tile([1, 128], BF16)
            nc.vector.memset(ones_row_bf, 1.0)
            ones_col2_bf = const.tile([128, 2], BF16)
            nc.vector.memset(ones_col2_bf, 1.0)
            ones8p = const.tile([128, 2, 16], FP8)
            nc.vector.memset(ones8p, 1.0)
            eps8_t = const.tile([128, 1], F32)
            nc.vector.memset(eps8_t, 1e-8)
            gamma_t = const.tile([128, 1], F32)
            nc.sync.dma_start(out=gamma_t, in_=a["gamma"].unsqueeze(0).to_broadcast([128, 1]))
            ball_row = const.tile([1, WA], BF16)
            lng_bc = const.tile([128, DC], F32)
            lnb_bc = const.tile([128, DC], F32)
            bq_col = const.tile([128, H], F32)
            bk_col = const.tile([128, H], F32)
            bor_row = const.tile([1, DC], BF16)
            bvr_row = const.tile([1, DC], BF16)

            cache_sb = cachep.tile([128, NT, DC], BF16)
            cache_f = cache_sb

            # ---------------- persistent attention tiles -------------------
            c2tb = attn.tile([128, 4, N], BF16)   # cache2T bf16 (DMA transpose)
            c2t = attn.tile([128, 4, N], FP8)
            v_sb = attn.tile([128, NT, DC], FP8)
            wq8 = attn.tile([128, 4, DC], FP8)
            wk8 = attn.tile([128, 4, DC], FP8)
            wv8 = attn.tile([128, 4, DC], FP8)
            wo8 = attn.tile([128, 4, DC], FP8)
            aoT = attn.tile([128, H, N], FP8)

            # gpsimd ring head: small casting bias DMAs (merged A bias row)
            wq3 = a["Wq"].rearrange("(c p) d -> p c d", p=128)
            wk3 = a["Wk"].rearrange("(c p) d -> p c d", p=128)
            wv3 = a["Wv"].rearrange("(c p) d -> p c d", p=128)
            wo3 = a["Wo"].rearrange("(c p) d -> p c d", p=128)
            base_t = LAYER_IDX * K // 128  # n-tile 6
            nc.gpsimd.dma_start(out=ball_row[:, 0:DC], in_=a["b_write"].unsqueeze(0))
            nc.gpsimd.dma_start(out=ball_row[:, DC:DC + K], in_=a["b_slot"].unsqueeze(0))
            nc.gpsimd.dma_start(out=ball_row[:, DC + K:DC + K + 1], in_=a["b_gate"].unsqueeze(0))
            nc.gpsimd.dma_start(out=ball_row[:, DC + K + 1:WA], in_=a["b_gate"].unsqueeze(0))
            nc.gpsimd.dma_start(out=bor_row, in_=a["bo"].unsqueeze(0))
            nc.gpsimd.dma_start(out=bvr_row, in_=a["bv"].unsqueeze(0))

            # ======================= phase A + B ===========================
            with (
                tc.tile_pool(name="wA", bufs=1) as wA,
                tc.tile_pool(name="pA", bufs=2) as pA,
                tc.tile_pool(name="pAs", bufs=3) as pAs,
                tc.tile_pool(name="pG", bufs=2) as pG,
                tc.tile_pool(name="psWV", bufs=2, space="PSUM") as psWV,
                tc.tile_pool(name="psLG", bufs=2, space="PSUM") as psLG,
                tc.tile_pool(name="psU", bufs=1, space="PSUM") as psU,
            ):
                # merged A weights [W_write | W_slot | W_gate W_gate]:
                # 4 fast f32 DMAs on the scalar ring, ACT casts -> fp8
                wall_f = wA.tile([128, DCH, WA], F32)
                wall8 = wA.tile([128, DCH, WA], FP8)
                wwr3 = a["W_write"].rearrange("(c p) d -> p c d", p=128)
                wsl3 = a["W_slot"].rearrange("(c p) k -> p c k", p=128)
                wgt3 = a["W_gate"].rearrange("(c p) o -> p c o", p=128)
                nc.scalar.dma_start(out=wall_f[:, :, 0:DC], in_=wwr3)
                nc.scalar.dma_start(out=wall_f[:, :, DC:DC + K], in_=wsl3)
                nc.scalar.dma_start(out=wall_f[:, :, DC + K:DC + K + 1], in_=wgt3)
                nc.scalar.dma_start(out=wall_f[:, :, DC + K + 1:WA], in_=wgt3)
                # scalar ring tail: broadcast constants (after the weights)
                nc.scalar.dma_start(out=bq_col, in_=a["bq"].rearrange("(h p) -> p h", p=128))
                nc.scalar.dma_start(out=bk_col, in_=a["bk"].rearrange("(h p) -> p h", p=128))
                nc.scalar.dma_start(out=lng_bc, in_=a["ln_g"].unsqueeze(0).to_broadcast([128, DC]))
                nc.scalar.dma_start(out=lnb_bc, in_=a["ln_b"].unsqueeze(0).to_broadcast([128, DC]))
                for cc in range(0, DCH, 2):
                    nc.scalar.copy(out=wall8[:, cc:cc + 2, :],
                                   in_=wall_f[:, cc:cc + 2, :])

                # prologue: first two y tiles DMA'd + cast (Pool)
                y_bf_pre = {}
                for i in range(2):
                    y_t = pA.tile([128, D], F32, tag="y", name=f"ypre{i}")
                    nc.sync.dma_start(out=y_t, in_=y3[:, i, :])
                    y_bf = pA.tile([128, D], BF16, tag="ybf", name=f"ybfpre{i}")
                    nc.gpsimd.tensor_copy(out=y_bf, in_=y_t)
                    y_bf_pre[i] = y_bf

                # gumbel Ln prepass: all Ln ops batched (one ACT table set)
                lnz_all = wA.tile([128, ST, K], F16)
                gum_h = {}
                for gg in range(2):
                    gum = pG.tile([128, 8, K], F32, tag="gum", name=f"gum{gg}")
                    nc.sync.dma_start(out=gum, in_=gum3[:, 8 * gg:8 * gg + 8, :])
                    gum_h[gg] = gum
                for g in range(8):
                    gum = gum_h[g // 4]
                    sl = gum[:, (g % 4) * 2:(g % 4) * 2 + 2, :]
                    lnu = pA.tile([128, 2, K], F32, tag="lnu")
                    nc.scalar.activation(lnu, sl, AF.Ln, bias=eps8_t)
                    nc.scalar.activation(lnz_all[:, 2 * g:2 * g + 2, :], lnu, AF.Ln,
                                         bias=eps8_t, scale=-1.0)

                # persistent scatter accumulators: per kc one [128, DC+2]
                # bank pair; cols DC:DC+2 accumulate the write mass (ones
                # columns folded into the rhs)
                ps_upd = [psU.tile([128, DC + 2], F32, name=f"upd{kc}", tag=f"upd{kc}")
                          for kc in range(2)]

                prev = [None]

                def flush_scatter():
                    if prev[0] is None:
                        return
                    j, w_j, wv_j = prev[0]
                    for kc in range(2):
                        lhs = w_j[:, kc * 128:(kc + 1) * 128]
                        nc.tensor.matmul(ps_upd[kc][:, 0:DC], lhs, wv_j,
                                         start=(j == 0), stop=(j == ST - 1))
                        nc.tensor.matmul(ps_upd[kc][:, DC:DC + 2], lhs,
                                         ones_col2_bf,
                                         start=(j == 0), stop=(j == ST - 1),
                                         skip_group_check=True)
                    prev[0] = None

                # cache tile pairs staged+cast in this order (phase B needs
                # tiles 6,7 first); each pair is DMA-transposed into c2tb
                # right after its cast (except the B-rewritten pair 6,7)
                cache_pairs = [(6, 7), (0, 1), (2, 3), (4, 5),
                               (8, 9), (10, 11), (12, 13), (14, 15)]
                y_pair = {}
                for i in range(ST):
                    if i == 2:
                        nc.gpsimd.dma_start(out=wv8, in_=wv3)
                    if i >= 2 and i % 2 == 0:
                        y_p = pA.tile([128, 2, D], F32, tag="y", name="ypair")
                        nc.sync.dma_start(out=y_p, in_=y3[:, i:i + 2, :])
                        y_pair[i] = y_p
                    if i in y_bf_pre:
                        y_bf = y_bf_pre[i]
                    else:
                        y_bf = pA.tile([128, D], BF16, tag="ybf")
                        nc.gpsimd.tensor_copy(out=y_bf, in_=y_pair[i - i % 2][:, i % 2, :])

                    # DMA-crossbar transpose -> yTb bf16, ACT cast -> fp8
                    yTb = pA.tile([128, DCH, 128], BF16, tag="yTb")
                    nc.scalar.dma_start_transpose(yTb, y_bf)
                    yT8 = pA.tile([128, DCH, 128], FP8, tag="yT8")
                    nc.scalar.copy(out=yT8, in_=yTb)

                    # write_vals / (logits, gate) DoubleRow matmuls (the
                    # moving AP's last dim is capped at 512 -> two regions)
                    ps_wv = psWV.tile([128, DC], F32, tag="wv")
                    ps_lg = psLG.tile([128, K + 2], F32, tag="lg")
                    for c in range(DCH // 2):
                        nc.tensor.matmul(
                            ps_wv, yT8[:, 2 * c:2 * c + 2, :],
                            wall8[:, 2 * c:2 * c + 2, 0:DC],
                            start=(c == 0), stop=False,
                            perf_mode=DR,
                        )
                    nc.tensor.matmul(ps_wv, ones_row_bf, ball_row[:, 0:DC],
                                     start=False, stop=True)
                    for c in range(DCH // 2):
                        nc.tensor.matmul(
                            ps_lg, yT8[:, 2 * c:2 * c + 2, :],
                            wall8[:, 2 * c:2 * c + 2, DC:WA],
                            start=(c == 0), stop=False,
                            perf_mode=DR,
                        )
                    nc.tensor.matmul(ps_lg, ones_row_bf, ball_row[:, DC:WA],
                                     start=False, stop=True)

                    # scatter matmuls for the previous tile (keeps PE dense
                    # while this tile's DVE/ACT chain runs)
                    flush_scatter()

                    # t = gamma*logits - lnz
                    t_sb = pAs.tile([128, K], F32, tag="tsb")
                    nc.vector.scalar_tensor_tensor(
                        out=t_sb, in0=ps_lg[:, 0:K], scalar=gamma_t,
                        in1=lnz_all[:, i, :], op0=ALU.mult, op1=ALU.subtract,
                    )
                    # scores = sigmoid(gate); s2 = scores / rowsum(exp(t))
                    sc_e = pAs.tile([128, 1], F32, tag="sce")
                    nc.scalar.activation(sc_e, ps_lg[:, K:K + 1],
                                         AF.Exp, scale=-1.0)
                    sc1 = pAs.tile([128, 1], F32, tag="sc1")
                    nc.vector.tensor_scalar_add(sc1, sc_e, 1.0)
                    p_un = pAs.tile([128, K], F32, tag="pun")
                    rs = pAs.tile([128, 1], F32, tag="rs")
                    nc.scalar.activation(p_un, t_sb, AF.Exp, accum_out=rs)
                    t3 = pAs.tile([128, 1], F32, tag="t3")
                    nc.vector.tensor_tensor(t3, sc1, rs, ALU.mult)
                    s2 = pAs.tile([128, 1], F32, tag="s2")
                    nc.vector.reciprocal(s2, t3)
                    w_sb = pAs.tile([128, K], BF16, tag="wsb")
                    nc.vector.tensor_scalar_mul(w_sb, p_un, s2)
                    # wv_sb = write_vals + b_write (bias folded via matmul)
                    wv_sb = pAs.tile([128, DC], BF16, tag="wvsb")
                    nc.vector.tensor_copy(out=wv_sb, in_=ps_wv)
                    prev[0] = (i, w_sb, wv_sb)

                    if i % 2 == 0:
                        ci0 = cache_pairs[i // 2][0]
                        cstg = pG.tile([128, 2, DC], F32, tag="cstg", name="cstg")
                        nc.gpsimd.dma_start(out=cstg, in_=cache3[:, ci0:ci0 + 2, :])
                        nc.vector.tensor_copy(out=cache_sb[:, ci0:ci0 + 2, :],
                                              in_=cstg)
                        if ci0 != base_t:
                            for t in (ci0, ci0 + 1):
                                nc.scalar.dma_start_transpose(
                                    c2tb[:, :, t * 128:(t + 1) * 128],
                                    cache_sb[:, t, :])

                flush_scatter()
                nc.gpsimd.dma_start(out=wk8, in_=wk3)
                nc.gpsimd.dma_start(out=wq8, in_=wq3)
                nc.gpsimd.dma_start(out=wo8, in_=wo3)

                # ------- phase B: slot update, overwrite cache rows -------
                for kc in range(2):
                    mass = pAs.tile([128, 1], F32, tag="mass")
                    nc.vector.tensor_copy(out=mass, in_=ps_upd[kc][:, DC:DC + 1])
                    m1 = pAs.tile([128, 1], F32, tag="m1")
                    nc.vector.tensor_scalar_add(m1, mass, EPS)
                    rm = pAs.tile([128, 1], F32, tag="rm")
                    nc.vector.reciprocal(rm, m1)
                    m2 = pAs.tile([128, 1], F32, tag="m2")
                    nc.vector.tensor_scalar_add(m2, mass, 1.0)
                    rg = pAs.tile([128, 1], F32, tag="rg")
                    nc.vector.reciprocal(rg, m2)
                    g_t = pAs.tile([128, 1], F32, tag="gt")
                    nc.vector.tensor_tensor(g_t, mass, rg, ALU.mult)
                    co = pAs.tile([128, 1], F32, tag="co")
                    nc.vector.tensor_scalar(co, g_t, -DECAY, DECAY, ALU.mult, ALU.add)
                    cn = pAs.tile([128, 1], F32, tag="cn")
                    nc.vector.tensor_tensor(cn, g_t, rm, ALU.mult)

                    told = pA.tile([128, DC], F32, tag="told")
                    nc.vector.tensor_scalar_mul(told, cache_f[:, base_t + kc, :], co)
                    nc.vector.scalar_tensor_tensor(
                        out=cache_sb[:, base_t + kc, :],
                        in0=ps_upd[kc][:, 0:DC], scalar=cn, in1=told,
                        op0=ALU.mult, op1=ALU.add,
                    )
                # phase C remainder: transpose the rewritten pair
                for t in (base_t, base_t + 1):
                    nc.scalar.dma_start_transpose(
                        c2tb[:, :, t * 128:(t + 1) * 128], cache_sb[:, t, :])

            # ============== phases C + D + E + F (shared PSUM) =============
            with (
                tc.tile_pool(name="attn2", bufs=1) as attn2,
                tc.tile_pool(name="pPT", bufs=2) as pPT,
                tc.tile_pool(name="psS", bufs=2, space="PSUM") as psS,
                tc.tile_pool(name="pEs", bufs=2) as pEs,
                tc.tile_pool(name="pF", bufs=2) as pF,
                tc.tile_pool(name="psDen", bufs=1, space="PSUM") as psDen,
                tc.tile_pool(name="psAo", bufs=1, space="PSUM") as psAo,
            ):
                # q/k staging + folded DoubleRow layouts (not live in phase A)
                qk8s = attn2.tile([128, 2, H, N], FP8)
                q8dr = attn2.tile([64, 2, H, N], FP8)  # [p,j,h,n] = q_h[n, 64j+p]
                k8dr = attn2.tile([64, 2, H, N], FP8)

                evac_flip = [0]

                def evac_copy(out_ap, in_ap):
                    if evac_flip[0] % 2 == 0:
                        nc.scalar.copy(out=out_ap, in_=in_ap)
                    else:
                        nc.vector.tensor_copy(out=out_ap, in_=in_ap)
                    evac_flip[0] += 1

                # cast cache2T -> fp8 (4 wide ops, alternating engines)
                for j in range(4):
                    evac_copy(c2t[:, j, :], c2tb[:, j, :])

                def d_slot(idx, name):
                    if idx % 3 == 2:
                        return psDen.tile([128, DC], F32, tag="den", name=name)
                    return psS.tile([128, DC], F32, tag="att", name=name)

                # ------- phase D: projections (fp8 DoubleRow) --------------
                slot_i = [0]
                for m in range(NT):
                    ps = d_slot(slot_i[0], f"v{m}"); slot_i[0] += 1
                    for g in range(2):
                        nc.tensor.matmul(
                            ps, c2t[:, 2 * g:2 * g + 2, m * 128:(m + 1) * 128],
                            wv8[:, 2 * g:2 * g + 2, :],
                            start=(g == 0), stop=False, perf_mode=DR,
                        )
                    nc.tensor.matmul(ps, ones_row_bf, bvr_row,
                                     start=False, stop=True)
                    evac_copy(v_sb[:, m, :], ps)
                for c in range(4):
                    for kq, (dst8, w8, b_col) in enumerate(
                            ((k8dr, wk8, bk_col), (q8dr, wq8, bq_col))):
                        for h in range(H):
                            ps = d_slot(slot_i[0], f"qk{c}_{h}_{kq}")
                            slot_i[0] += 1
                            for g in range(2):
                                nc.tensor.matmul(
                                    ps, w8[:, 2 * g:2 * g + 2, h * 128:(h + 1) * 128],
                                    c2t[:, 2 * g:2 * g + 2, c * 512:(c + 1) * 512],
                                    start=(g == 0), stop=(g == 1), perf_mode=DR,
                                )
                            if evac_flip[0] % 2 == 0:
                                nc.scalar.activation(
                                    qk8s[:, kq, h, c * 512:(c + 1) * 512], ps,
                                    AF.Identity, bias=b_col[:, h:h + 1])
                            else:
                                nc.vector.tensor_scalar_add(
                                    qk8s[:, kq, h, c * 512:(c + 1) * 512], ps,
                                    b_col[:, h:h + 1])
                            evac_flip[0] += 1
                        # partition fold for DoubleRow QK (SBUF->SBUF DMA)
                        for jj in range(2):
                            nc.scalar.dma_start(
                                out=dst8[:, jj, :, c * 512:(c + 1) * 512],
                                in_=qk8s[64 * jj:64 * jj + 64, kq, :,
                                         c * 512:(c + 1) * 512],
                            )

                def emit_F_pair(c, pr, half, st):
                    # o-proj + LN stats for n-tile 4c+2*pr+half
                    if half == 0:
                        st["mean2"] = pF.tile([128, 2], F32, tag="mean2",
                                              name=f"mean2_{c}_{pr}")
                        st["var2"] = pF.tile([128, 2], F32, tag="var2",
                                             name=f"var2_{c}_{pr}")
                        st["r"] = []
                    mean2, var2 = st["mean2"], st["var2"]
                    t = 4 * c + 2 * pr + half
                    ps_o = psS.tile([128, DC], F32, tag="att",
                                    name=f"o{c}_{pr}_{half}")
                    for g in range(2):
                        nc.tensor.matmul(
                            ps_o, aoT[:, 2 * g:2 * g + 2, t * 128:(t + 1) * 128],
                            wo8[:, 2 * g:2 * g + 2, :],
                            start=(g == 0), stop=False, perf_mode=DR,
                        )
                    nc.tensor.matmul(ps_o, ones_row_bf, bor_row,
                                     start=False, stop=True)
                    r_t = pF.tile([128, DC], F32, tag="r")
                    rsum = pF.tile([128, 1], F32, tag="rsum")
                    nc.vector.scalar_tensor_tensor(
                        out=r_t, in0=ps_o, scalar=1.0,
                        in1=cache_f[:, t, :],
                        op0=ALU.mult, op1=ALU.add, accum_out=rsum,
                    )
                    nc.vector.tensor_scalar_mul(
                        mean2[:, half:half + 1], rsum, 1.0 / DC)
                    scratch = pF.tile([128, DC], F32, tag="scratch")
                    nc.vector.scalar_tensor_tensor(
                        out=scratch, in0=r_t,
                        scalar=mean2[:, half:half + 1], in1=r_t,
                        op0=ALU.subtract, op1=ALU.mult,
                        accum_out=var2[:, half:half + 1],
                    )
                    st["r"].append(r_t)

                def emit_F_tail(c, pr, st):
                    mean2, var2, r_ts = st["mean2"], st["var2"], st["r"]
                    # rstd = 1/sqrt(var/DC + 1e-5), quake + 2 Newton (DVE only)
                    nc.vector.tensor_scalar(var2, var2, 1.0 / DC, 1e-5,
                                            ALU.mult, ALU.add)
                    vh = pF.tile([128, 2], F32, tag="vh")
                    nc.vector.tensor_scalar_mul(vh, var2, -0.5)
                    qi = pF.tile([128, 2], I32, tag="qi")
                    nc.vector.tensor_scalar(qi, var2.bitcast(I32), 1, -1,
                                            ALU.logical_shift_right, ALU.bitwise_xor)
                    rstd = pF.tile([128, 2], F32, tag="rstd")
                    nc.vector.tensor_scalar_add(rstd.bitcast(I32), qi, QMAGIC)
                    yy = pF.tile([128, 2], F32, tag="yy")
                    for _ in range(2):
                        nc.vector.tensor_tensor(yy, rstd, rstd, ALU.mult)
                        nc.vector.tensor_tensor(yy, yy, vh, ALU.mult)
                        nc.vector.tensor_scalar_add(yy, yy, 1.5)
                        nc.vector.tensor_tensor(rstd, rstd, yy, ALU.mult)
                    for half in range(2):
                        t = 4 * c + 2 * pr + half
                        t1 = pF.tile([128, DC], F32, tag="t1")
                        nc.vector.tensor_scalar(
                            t1, r_ts[half], mean2[:, half:half + 1],
                            rstd[:, half:half + 1], ALU.subtract, ALU.mult)
                        t2 = pF.tile([128, DC], F32, tag="t2")
                        nc.vector.scalar_tensor_tensor(
                            out=t2, in0=t1, scalar=1.0, in1=lng_bc,
                            op0=ALU.mult, op1=ALU.mult,
                        )
                        o_sb = pF.tile([128, DC], F32, tag="osb")
                        nc.vector.scalar_tensor_tensor(
                            out=o_sb, in0=t2, scalar=1.0, in1=lnb_bc,
                            op0=ALU.mult, op1=ALU.add,
                        )
                        nc.sync.dma_start(out=out3[:, t, :], in_=o_sb)

                f_state = {}
                prev_ps = [None]

                def finish_pass(st):
                    # 1/den via one Newton step from y0=2^-11 (den ~ 2048),
                    # broadcast den rows, fused normalize evac
                    cc, ph = st["c"], st["ph"]
                    den_row = pEs.tile([1, CL * 2], F32, tag="denrow",
                                       name=f"denrow{cc}_{ph}")
                    nc.vector.tensor_copy(out=den_row, in_=st["den"][0:1, :])
                    bc_den = pEs.tile([128, CL * 2], F32, tag="bcden",
                                      name=f"bcden{cc}_{ph}")
                    nc.gpsimd.partition_broadcast(bc_den, den_row)
                    Y0 = 1.0 / 2048.0
                    nc.vector.tensor_scalar(bc_den, bc_den, -Y0, 2.0,
                                            ALU.mult, ALU.add)
                    nc.vector.tensor_scalar_mul(bc_den, bc_den, Y0)
                    for hh in range(2):
                        h = 2 * ph + hh
                        nc.vector.scalar_tensor_tensor(
                            out=aoT[:, h, cc * CL:(cc + 1) * CL],
                            in0=st["ao"][hh],
                            scalar=1.0, in1=bc_den[:, hh * CL:(hh + 1) * CL],
                            op0=ALU.mult, op1=ALU.mult,
                        )

                def emit_pair(st, j, last):
                    # PV fp8 DoubleRow over m-tile pair {2j, 2j+1} +
                    # one h-paired DoubleRow denominator matmul
                    ph, pT = st["ph"], st["pT"]
                    for hh in range(2):
                        h = 2 * ph + hh
                        nc.tensor.matmul(
                            st["ao"][hh],
                            v_sb[:, 2 * j:2 * j + 2, h * 128:(h + 1) * 128],
                            pT[:, hh, 2 * j:2 * j + 2, :],
                            start=(j == 0), stop=last,
                            perf_mode=DR,
                        )
                    for hh in range(2):
                        nc.tensor.matmul(
                            st["den"][0:1, hh * CL:(hh + 1) * CL],
                            ones8p[:, :, 0:1],
                            pT[:, hh, 2 * j:2 * j + 2, :],
                            start=(j == 0), stop=last,
                            perf_mode=DR,
                            skip_group_check=True,
                        )

                for c in range(NCH):
                    for ph in range(2):
                        ps_aoA = psAo.tile([128, CL], F32, name=f"aoA{c}_{ph}", tag="aoA")
                        ps_aoB = psAo.tile([128, CL], F32, name=f"aoB{c}_{ph}", tag="aoB")
                        ps_den = psDen.tile([128, 2 * CL], F32, name=f"den{c}_{ph}", tag="den")
                        pT = pPT.tile([128, 2, NT, CL], FP8, tag="pT", name=f"pT{c}_{ph}")
                        cur = {"c": c, "ph": ph, "ao": [ps_aoA, ps_aoB],
                               "den": ps_den, "pT": pT}

                        for m in range(NT):
                            ps_a = psS.tile([128, 2 * CL], F32, tag="att")
                            for hh in range(2):
                                h = 2 * ph + hh
                                nc.tensor.matmul(
                                    ps_a[:, hh * CL:(hh + 1) * CL],
                                    k8dr[:, :, h, m * 128:(m + 1) * 128],
                                    q8dr[:, :, h, c * CL:(c + 1) * CL],
                                    start=True, stop=True,
                                    perf_mode=DR,
                                )
                            eng = EXP_ENG[m]
                            pslc = pT[:, :, m, :]
                            ps_a3 = ps_a.rearrange("p (h n) -> p h n", h=2)
                            if eng == 'a':
                                nc.scalar.activation(pslc, ps_a3,
                                                     AF.Exp, scale=ATT_SCALE)
                            elif eng == 'v':
                                nc.vector.tensor_scalar(
                                    pslc.bitcast(I8), ps_a3,
                                    SCH_A, SCH_B, ALU.mult, ALU.add)
                            else:
                                nc.gpsimd.tensor_scalar(
                                    pslc.bitcast(I8), ps_a3,
                                    SCH_A, SCH_B, ALU.mult, ALU.add)
                            pv = prev_ps[0]
                            if pv is not None:
                                if m == 0:
                                    emit_pair(pv, 7, True)
                                elif m == 1:
                                    finish_pass(pv)
                                elif c > 0:
                                    if m == 5:
                                        emit_F_pair(c - 1, ph, 0, f_state)
                                    elif m == 6:
                                        emit_F_pair(c - 1, ph, 1, f_state)
                                    elif m == 7:
                                        emit_F_tail(c - 1, ph, f_state)
                            if m >= 9:
                                emit_pair(cur, m - 9, False)
                        prev_ps[0] = cur

                emit_pair(prev_ps[0], 7, True)
                finish_pass(prev_ps[0])
                for pr in range(2):
                    emit_F_pair(NCH - 1, pr, 0, f_state)
                    emit_F_pair(NCH - 1, pr, 1, f_state)
                    emit_F_tail(NCH - 1, pr, f_state)

    nc.compile()
    return nc


_NC_CACHE = {}


def _get_nc():
    if "nc" not in _NC_CACHE:
        _NC_CACHE["nc"] = _build()
    return _NC_CACHE["nc"]


def _in_maps(inputs):
    per_batch = {"y", "cache", "gumbel_u"}
    maps = []
    for b in range(B):
        m = {}
        for name in _INPUT_SPECS:
            arr = np.ascontiguousarray(np.asarray(inputs[name], dtype=np.float32))
            m[name] = arr[b] if name in per_batch else arr
        maps.append(m)
    return maps


def _execute(inputs, trace=False):
    nc = _get_nc()
    res = run_bass_kernel_spmd(nc, _in_maps(inputs), list(range(B)), trace=trace)
    out = np.stack([res.results[b]["out"] for b in range(B)]).astype(np.float32)
    return out, res


def kernel(**inputs) -> np.ndarray:
    out, _ = _execute(inputs)
    return out
